# revision 2
# baseline (speedup 1.0000x reference)
"""Trainium2 Bass kernel for the HFNN forward pass (v4: fp16 membership).

Math (per branch k of 8, rule r of 32, feature f of 16, batch b of 32768):
  expo[k,b,r] = sum_f (a_neg*x^2 + m2*x) - c ;  E = exp(expo)
  G[k,f,b]    = sum_r E * w3[k,r,f]
  den[k,b]    = sum_r E ; numb[k,b] = sum_r E*w3bias
  num0[k,b]   = sum_f x * G
  tsk = (num0+numb)/den ; out = softmax over 2 classes (host, f64)

Device strategy (batch data-parallel over 8 cores, 4096 each):
  - Host ships two fp16 slabs per core: y0 = [x br0-3 ; x^2 br0-3] and
    y1 = [x^2 br4-7 ; x br4-7] (row 16k+f within each 64-half), so the
    membership is ONE K=128 fp16 matmul per group per 512-chunk.
  - exp on ACT at 1024 wide (psum m tiles, fp32 bias rides w16 bit-packed
    as fp16 pairs), E fp32.
  - G psum per chunk: accumulate wG0^T E0 + wG1^T E1 (zero-padded lhsT).
  - xg = x .* G: row half 0-63 on DVE, 64-127 on Pool; the last super uses
    a dedicated x-full slab so single muls feed the tail.
  - collector psum rows 24q+{den,numb,num0} via M=128 col-sparse passes.
  - PE emission is software-pipelined one super deep:
      [mem(s)] [red(s-1)] [GA(s)] [num0(s-1)+out(s-1)] [GB(s)]
    so every deferred pass has its deps ready when PE reaches it.
  - PE p-state warmup matmuls + ACT exp-table warmup at t~0.
"""

import numpy as np

import concourse.bacc as bacc
import concourse.tile as tile
from concourse import mybir
from concourse.bass_utils import run_bass_kernel_spmd

F32 = mybir.dt.float32
F32R = mybir.dt.float32r
F16 = mybir.dt.float16
BF16 = mybir.dt.bfloat16

NB, NR, NF = 8, 32, 16
NBATCH, NCORE = 32768, 8
BC = NBATCH // NCORE          # 4096 batch per core
CH = 512                      # chunk width (psum bank)
SUP = 1024                    # superchunk width
NSUP = BC // SUP              # 4 supers == collector rounds

_CACHE: dict = {}


def _build_nc():
    nc = bacc.Bacc("TRN2", target_bir_lowering=False, debug=False)
    y0_in = nc.dram_tensor("y0", [128, BC], F16, kind="ExternalInput")
    y1_in = nc.dram_tensor("y1", [128, BC], F16, kind="ExternalInput")
    xl_in = nc.dram_tensor("xl", [128, 3 * SUP], F16, kind="ExternalInput")
    # fp16 weights: mem lhsT g0 | g1 | num0 q0 | q1 (4 x [128,128])
    w16_in = nc.dram_tensor("w16", [128, 512], F16, kind="ExternalInput")
    # fp32 weights: bias(2) | pad(2) | wG0ext | wG1ext | red00 .. red11
    w32_in = nc.dram_tensor("w32", [128, 772], F32R, kind="ExternalInput")
    out_c = nc.dram_tensor("outc", [NSUP, 48, CH], F32, kind="ExternalOutput")

    with tile.TileContext(nc) as tc:
        with (
            tc.tile_pool(name="wpool", bufs=1) as wpool,
            tc.tile_pool(name="ypool", bufs=10) as ypool,
            tc.tile_pool(name="epool", bufs=4) as epool,
            tc.tile_pool(name="xgpool", bufs=4) as xgpool,
            tc.tile_pool(name="opool", bufs=3) as opool,
            tc.tile_pool(name="mps", bufs=2, space="PSUM") as mps,
            tc.tile_pool(name="gps", bufs=2, space="PSUM") as gps,
            tc.tile_pool(name="cps", bufs=2, space="PSUM") as cps,
        ):
            # ---- warmup + weight prologue ----
            # w16 rides SWDGE (Pool's first op) so it doesn't take an HWDGE
            # slot ahead of the first input chunk.
            w16 = wpool.tile([128, 512], F16, tag="w16")
            nc.gpsimd.dma_start(out=w16[:], in_=w16_in[:, :])
            # PE p-state ramp needs ~3us of queued matmul activity before the
            # first real pass; feed it dummy N=256 matmuls from t~0.5.
            warm_rf = wpool.tile([128, 256], F32, tag="warmr")
            nc.vector.memset(warm_rf[:], 0.0)
            warm_r = warm_rf[:].bitcast(F32R)
            # ACT: force the exp table load now (1283 ns).
            warm_a = wpool.tile([128, 1], F32, tag="warma")
            nc.vector.memset(warm_a[:], 0.0)
            nc.scalar.activation(warm_a[:], warm_a[:], mybir.ActivationFunctionType.Exp)

            wps = cps.tile([128, CH], F32, tag="coll", name="warmps")
            for _ in range(12):
                nc.tensor.matmul(
                    wps[0:1, 0:256], warm_r[:, 0:1], warm_r[:, :], start=True, stop=True
                )

            # ---- input DMAs (sync queue -> HWDGE) ----
            y0s0a = ypool.tile([128, CH], F16, tag="y", name="y0s0a")
            y0s0b = ypool.tile([128, CH], F16, tag="y", name="y0s0b")
            w32a = wpool.tile([128, 260], F32R, tag="w32a")
            nc.sync.dma_start(out=y0s0a[:], in_=y0_in[:, 0:CH])
            nc.sync.dma_start(out=w32a[:], in_=w32_in[:, 0:260])
            nc.sync.dma_start(out=y0s0b[:], in_=y0_in[:, CH:SUP])
            y_t = {}
            for s in range(NSUP):
                for sl in range(2):
                    if s == 0 and sl == 0:
                        continue
                    y_t[(sl, s)] = ypool.tile(
                        [128, SUP], F16, tag="y", name=f"y{sl}s{s}"
                    )
            xl_a = ypool.tile([128, SUP + CH], F16, tag="y", name="xla")
            xl_b = ypool.tile([128, SUP + CH], F16, tag="y", name="xlb")
            w32b = wpool.tile([128, 512], F32R, tag="w32b")
            nc.sync.dma_start(out=y_t[(1, 0)][:], in_=y1_in[:, 0:SUP])
            nc.sync.dma_start(out=y_t[(0, 1)][:], in_=y0_in[:, SUP : 2 * SUP])
            nc.sync.dma_start(out=y_t[(1, 1)][:], in_=y1_in[:, SUP : 2 * SUP])
            nc.sync.dma_start(out=w32b[:], in_=w32_in[:, 260:772])
            for s in range(2, NSUP):
                nc.sync.dma_start(out=y_t[(0, s)][:], in_=y0_in[:, s * SUP : (s + 1) * SUP])
                nc.sync.dma_start(out=y_t[(1, s)][:], in_=y1_in[:, s * SUP : (s + 1) * SUP])
            nc.sync.dma_start(out=xl_a[:], in_=xl_in[:, 0 : SUP + CH])
            nc.sync.dma_start(out=xl_b[:], in_=xl_in[:, SUP + CH : 3 * SUP])

            def xlchunk(s, h):
                """x-full rows for super s >= 1, chunk h."""
                off = (s - 1) * SUP + h * CH
                if off < SUP + CH:
                    return xl_a, slice(off, off + CH)
                off -= SUP + CH
                return xl_b, slice(off, off + CH)

            def ychunk(sl, s, h):
                """(tile, col slice) holding slab sl, super s, chunk-half h."""
                if s == 0 and sl == 0:
                    return (y0s0a if h == 0 else y0s0b), slice(0, CH)
                return y_t[(sl, s)], slice(h * CH, (h + 1) * CH)

            def wmem(g):
                return w16[:, 128 * g : 128 * (g + 1)]

            def wnum0(q):
                # bf16 bit-patterns stored in the fp16 tensor (xg is bf16 so
                # its fp16-range underflow at tiny memberships is avoided)
                return w16[:, 256 + 128 * q : 384 + 128 * q].bitcast(BF16)

            def wG(g):
                return w32a[:, 4 + 128 * g : 4 + 128 * (g + 1)]

            def wred(g, q):
                i = 2 * g + q
                return w32b[:, 128 * i : 128 * (i + 1)]

            bias_ap = w32a[:, 0:2].bitcast(F32)

            # ---- pipeline stages ----
            state = {}

            def emit_A(s, gorder):
                m_t = [mps.tile([128, SUP], F32, tag="m", name=f"m{s}g{g}") for g in range(2)]
                e_t = [
                    epool.tile([128, SUP], F32R, tag="e", name=f"e{s}g{g}")
                    for g in range(2)
                ]
                for g in gorder:
                    for h in range(2):
                        yt, cs = ychunk(g, s, h)
                        nc.tensor.matmul(
                            m_t[g][:, h * CH : (h + 1) * CH], wmem(g), yt[:, cs],
                            start=True, stop=True,
                        )
                    nc.scalar.activation(
                        e_t[g][:], m_t[g][:],
                        mybir.ActivationFunctionType.Exp,
                        bias=bias_ap[:, g : g + 1], scale=1.0,
                    )
                state[s] = {"e": e_t}

            def emit_red(s, colls, gorder=(0, 1)):
                """den/numb passes for super s into its collector(s)."""
                e_t = state[s]["e"]
                state[s]["coll"] = colls
                started = set()
                for h in range(2):
                    cl = colls[h] if len(colls) == 2 else colls[0]
                    for g in gorder:
                        nc.tensor.matmul(
                            cl[:], wred(g, h), e_t[g][:, h * CH : (h + 1) * CH],
                            start=(id(cl) not in started), stop=False,
                            skip_group_check=True,
                        )
                        started.add(id(cl))

            def emit_G(s, gfirst, h):
                e_t = state[s]["e"]
                g_ps = state[s].setdefault("g", {})
                g_ps[h] = gps.tile([128, CH], F32, tag="g", name=f"g{s}h{h}")
                nc.tensor.matmul(
                    g_ps[h][:], wG(gfirst), e_t[gfirst][:, h * CH : (h + 1) * CH],
                    start=True, stop=False,
                )

            def emit_G2_xg(s, gsecond, h):
                e_t = state[s]["e"]
                g_ps = state[s]["g"]
                nc.tensor.matmul(
                    g_ps[h][:], wG(gsecond), e_t[gsecond][:, h * CH : (h + 1) * CH],
                    start=False, stop=True,
                )
                xg = state[s].setdefault("xg", {})
                xg[h] = xgpool.tile([128, CH], BF16, tag="xg", name=f"xg{s}h{h}")
                if s > 0:
                    xt, cs = xlchunk(s, h)
                    nc.vector.tensor_mul(xg[h][:, :], xt[:, cs], g_ps[h][:, :])
                else:
                    # GPSIMD cannot touch PSUM: both halves ride DVE (early,
                    # DVE has slack in super 0)
                    yt0, cs0 = ychunk(0, s, h)
                    yt1, cs1 = ychunk(1, s, h)
                    nc.vector.tensor_mul(
                        xg[h][0:64, :], yt0[0:64, cs0], g_ps[h][0:64, :]
                    )
                    nc.vector.tensor_mul(
                        xg[h][64:128, :], yt1[64:128, cs1], g_ps[h][64:128, :]
                    )

            def emit_num0_out(s):
                colls = state[s]["coll"]
                xg = state[s]["xg"]
                if len(colls) == 1:
                    coll = colls[0]
                    for h in range(2):
                        nc.tensor.matmul(
                            coll[:], wnum0(h), xg[h][:],
                            start=False, stop=(h == 1), skip_group_check=True,
                        )
                    ot = opool.tile([128, CH], F32, tag="o", name=f"o{s}")
                    nc.vector.tensor_copy(ot[0:48, :], coll[0:48, :])
                    nc.sync.dma_start(out=out_c[s], in_=ot[0:48, :])
                else:
                    for h in range(2):
                        nc.tensor.matmul(
                            colls[h][:], wnum0(h), xg[h][:],
                            start=False, stop=True, skip_group_check=True,
                        )
                        ot = opool.tile([128, CH], F32, tag="o", name=f"o{s}q{h}")
                        rs = slice(24 * h, 24 * h + 24)
                        nc.scalar.copy(ot[0:48, :], colls[h][0:48, :])
                        nc.sync.dma_start(out=out_c[s][rs], in_=ot[rs, :])

            # ---- schedule ----
            # steady supers 0..2: PE stream per iter s:
            #   [mem(s)x4+exps] [red(s-1)x4] [GA(s)x2] [num0(s-1)x2+out] [GB(s)x2]
            for s in range(NSUP - 1):
                emit_A(s, (0, 1))
                if s > 0:
                    emit_red(s - 1, [cps.tile([128, CH], F32, tag="coll", name=f"coll{s-1}")])
                    emit_G(s - 1, 0, 0)
                    emit_G(s - 1, 0, 1)
                if s > 1:
                    emit_num0_out(s - 2)
                if s > 0:
                    emit_G2_xg(s - 1, 1, 0)
                    emit_G2_xg(s - 1, 1, 1)
            # last super: g1 first so exp(g1) lands early; G accumulates
            # GB-then-GA so the G psum closes as soon as exp(g0) is done.
            s = NSUP - 1
            emit_A(s, (1, 0))
            emit_red(s - 1, [cps.tile([128, CH], F32, tag="coll", name=f"coll{s-1}")])
            emit_G(s - 1, 0, 0)
            emit_G(s - 1, 0, 1)
            emit_num0_out(s - 2)
            emit_G2_xg(s - 1, 1, 0)
            emit_G2_xg(s - 1, 1, 1)
            # drain: last super inline, tail-chain optimized
            collA = cps.tile([128, CH], F32, tag="coll", name="collA")
            collB = cps.tile([128, CH], F32, tag="coll", name="collB")
            emit_G(s, 1, 0)          # GB(h0) right after exp(g1)
            emit_G(s, 1, 1)
            emit_red(s, [collA, collB], gorder=(1, 0))
            emit_num0_out(s - 1)
            emit_G2_xg(s, 0, 0)      # GA closes G as soon as exp(g0) lands
            emit_G2_xg(s, 0, 1)
            emit_num0_out(s)
    nc.finalize()
    return nc


def _host_prep(data, para_mu, para_sigma, para_w3):
    xt = np.ascontiguousarray(data.transpose(0, 2, 1)).astype(np.float64)
    xslab = xt.reshape(128, NBATCH)                 # row 16k+f
    x2slab = xslab * xslab
    y0 = np.empty((128, NBATCH), np.float16)
    y1 = np.empty((128, NBATCH), np.float16)
    y0[0:64] = xslab[0:64]                          # x br0-3
    y0[64:128] = x2slab[0:64]                       # x^2 br0-3
    y1[0:64] = x2slab[64:128]                       # x^2 br4-7
    y1[64:128] = xslab[64:128]                      # x br4-7
    xl = xslab.astype(np.float16)                   # full x rows, supers 1-3

    sig2 = para_sigma.astype(np.float64) ** 2
    mu = para_mu.astype(np.float64)
    a_neg = -1.0 / (2.0 * sig2)                     # [8, 32, 16]
    m2 = mu / sig2
    c = np.sum(mu * mu / (2.0 * sig2), axis=-1)     # [8, 32]

    # fp16 weights: mem lhsT per group + num0 blocks
    w16 = np.zeros((128, 512), np.float16)
    for i in range(4):
        rows_lo = slice(16 * i, 16 * i + 16)        # partitions 0-63 block i
        rows_hi = slice(64 + 16 * i, 64 + 16 * i + 16)
        cols = slice(32 * i, 32 * i + 32)
        # g0: y0 = [x ; x^2]  -> rows 0-63 m2, rows 64-127 a_neg (k = i)
        w16[rows_lo, cols] = m2[i].T
        w16[rows_hi, cols] = a_neg[i].T
        # g1: y1 = [x^2 ; x]  -> rows 0-63 a_neg, rows 64-127 m2 (k = 4+i)
        w16[rows_lo, 128 + 32 * i : 128 + 32 * i + 32] = a_neg[4 + i].T
        w16[rows_hi, 128 + 32 * i : 128 + 32 * i + 32] = m2[4 + i].T
    w16u = w16.view(np.uint16)
    for q in range(2):
        for k in range(NB):
            # num0: contract xg rows 16k+f -> col 24q+16+k; the matmul reads
            # these cols as bf16, so store bf16(1.0) bit patterns
            w16u[16 * k : 16 * k + 16, 256 + 128 * q + 24 * q + 16 + k] = 0x3F80
    # layout: [bias(2) | pad(2) | wG0 | wG1 | red00 | red01 | red10 | red11]
    w32 = np.zeros((128, 772), np.float32)
    for g in range(2):
        for i in range(4):
            k = 4 * g + i
            rrows = slice(32 * i, 32 * i + 32)      # E_g partition rows
            w32[rrows, g] = -c[k]                   # exp bias -c per rule row
            # wGext: E_g rows -> G cols 16k+f (g1 cols land at 64-127)
            w32[rrows, 4 + 128 * g + 16 * k : 4 + 128 * g + 16 * k + 16] = (
                para_w3[k, :, :NF]
            )
            for q in range(2):
                blk = 260 + 128 * (2 * g + q)
                w32[rrows, blk + 24 * q + k] = 1.0                  # den
                w32[rrows, blk + 24 * q + 8 + k] = para_w3[k, :, NF]  # numb
    return y0, y1, xl, w16, w32


def kernel(data, para_mu, para_sigma, para_w3, w5, b5):
    if "nc" not in _CACHE:
        _CACHE["nc"] = _build_nc()
    nc = _CACHE["nc"]

    y0, y1, xl, w16, w32 = _host_prep(data, para_mu, para_sigma, para_w3)
    in_maps = []
    for cidx in range(NCORE):
        cols = slice(cidx * BC, (cidx + 1) * BC)
        in_maps.append(
            {
                "y0": np.ascontiguousarray(y0[:, cols]),
                "y1": np.ascontiguousarray(y1[:, cols]),
                "xl": np.ascontiguousarray(xl[:, cidx * BC + SUP : (cidx + 1) * BC]),
                "w16": w16,
                "w32": w32,
            }
        )
    try:
        res = run_bass_kernel_spmd(nc, in_maps, core_ids=list(range(NCORE)))
    except Exception:
        # transient NRT device errors (e.g. a wedged core) recover on retry
        res = run_bass_kernel_spmd(nc, in_maps, core_ids=list(range(NCORE)))
    _CACHE["last_result"] = res

    # ---- host epilogue (exact, O(B)) ----
    den = np.empty((NB, NBATCH), np.float64)
    numb = np.empty((NB, NBATCH), np.float64)
    num0 = np.empty((NB, NBATCH), np.float64)
    for cidx in range(NCORE):
        arr = res.results[cidx]["outc"].astype(np.float64)  # [4, 48, 512]
        v = np.moveaxis(arr.reshape(NSUP, 2, 24, CH), 2, 0)
        v = v.reshape(24, BC)  # row l, local batch (rnd, q, t)
        cols = slice(cidx * BC, (cidx + 1) * BC)
        den[:, cols] = v[0:8]
        numb[:, cols] = v[8:16]
        num0[:, cols] = v[16:24]

    tsk = (num0 + numb) / den                     # [8, B]
    w5d = (w5[0] - w5[1]).astype(np.float64)
    d = w5d @ tsk + (float(b5[0]) - float(b5[1]))
    p0 = 1.0 / (1.0 + np.exp(-d))
    out = np.empty((NBATCH, 2), np.float32)
    out[:, 0] = p0.astype(np.float32)
    out[:, 1] = (1.0 - p0).astype(np.float32)
    return out
